# revision 67
# baseline (speedup 1.0000x reference)
"""HTM spatial-pooler kernel for Trainium2 (8 NeuronCores, data-parallel over tokens).

Computes, for x = input_vector reshaped to [4096 tokens, 4096]:
    overlap = x @ C^T               (C = connections [2048, 4096], binary)
    boosted = overlap * boost       (per-column boosting factors)
    masked  = where(boosted >= kth_largest_per_row(boosted, k), boosted, 0)

Strategy per core (512 tokens, 4 m-tiles of 128):
  - Matmul in fp8 (e4m3) with MatmulPerfMode.DoubleRow: each PE instruction
    contracts 256 (two 128-chunks) at 0.5 cycles/row -> 4x bf16 throughput.
  - Precision: x is centered (y = x - 0.5) and split y ~= hi + lo * 2^-6 with
    hi = e4m3(y), lo = e4m3((y - hi) * 64).  The lo pass accumulates into its
    own PSUM bank and is scaled by 2^-6 on the Activation engine while being
    moved to SBUF, then added to the hi PSUM on the DVE.  This gives ~7e-3
    score noise on scores whose top-40/41 gap is ~0.77 -> a handful of
    flipped rows (well inside the 2e-2 rel-err gate).
  - The centering constant (0.5 * popcount(C_k) per column) rides inside the
    matmul as an 18th DoubleRow pair: x-side is a constant column (4.0 for
    the hi pass, 1.0 for the lo pass) on partition 0, ct-side holds a 4-term
    e4m3 cascade encoding of the constant, so no extra DVE pass is needed.
  - Top-k threshold per row on the DVE (per-64-col top-8 candidates via
    max8, then max8/match_replace rounds; m0/m1/m3 use a pre-top-k over the
    first half plus a short merge round to shorten the critical tail).
    Masks are (boosted >= thr) * boosted written as bf16: one DVE
    scalar_tensor_tensor for m0/m1/m3, and an exact ACT+Pool construction
    (bo - bo * sign(relu(thr - bo))) for m2 to keep the DVE tail clear.
  - Engine budget: PE matmuls; ACT drains/rescales PSUM; Pool (GPSIMD,
    which can't touch PSUM and lacks TensorScalarPtr) takes the SBUF-side
    add/boost-multiply for non-critical chunks; DVE keeps the top-k and the
    critical-path combines.
  - Phase schedule: m0+m1 run column-half-at-a-time with j outermost so the
    PE consumes each ct pair-chunk as its DMA lands (ct streams 9.4 MB);
    m2/m3 run as j-outer pair groups / j-inner chunks with ct resident on
    alternating PSUM bank sets, completing chunk-eagerly so the
    combine/top-k pipeline hides behind the PE.
"""
import math

import numpy as np
import ml_dtypes

import concourse.bacc as bacc
import concourse.mybir as mybir
from concourse import tile
from concourse.bass_utils import run_bass_kernel_spmd

FP8 = mybir.dt.float8e4
BF16 = mybir.dt.bfloat16
F32 = mybir.dt.float32
E4M3 = ml_dtypes.float8_e4m3

N_CORES = 8
TOK_PER_CORE = 512
M_TILES = 4          # 128-token tiles per core
D = 4096             # input size (contraction)
KC = D // 128        # 32 contraction chunks
NPAIR = KC // 2      # 16 DoubleRow pairs of x data
NJ = NPAIR + 1       # +1 rs-constant pair
NCOL = 2048          # minicolumns
HALF = NCOL // 2
LO_SCALE = 2.0 ** -6

C3_ENG = [lambda nc: nc.gpsimd]  # Pool TT lowers to GPSIMD Multiply

_BUILD_CACHE = {}


def _build(k_active: int):
    nc = bacc.Bacc("TRN2", target_bir_lowering=False)
    DR = mybir.MatmulPerfMode.DoubleRow
    ADD = mybir.AluOpType.add
    MULT = mybir.AluOpType.mult
    ISGE = mybir.AluOpType.is_ge

    # x: per (m, pass): pairs j=0..16 (j=16 = rs constant column), split into
    # an early tile (j<9) and a late tile so the PE can start after ~3.3 us.
    JA, JB = 9, NJ - 9
    xhi = nc.dram_tensor("xhi", [M_TILES, 128, NJ, 2, 128], FP8, kind="ExternalInput")
    xlo = nc.dram_tensor("xlo", [M_TILES, 128, NJ, 2, 128], FP8, kind="ExternalInput")
    # ct: [half, jj, ks, 2*1024]; jj=16 -> hi-pass rs rows, jj=17 -> lo-pass
    ct = nc.dram_tensor("ct", [2, 128, NJ + 1, 2, 1024], FP8, kind="ExternalInput")
    bc = nc.dram_tensor("bc", [4, 128, 512], F32, kind="ExternalInput")
    out = nc.dram_tensor("out", [M_TILES, 128, NCOL], BF16, kind="ExternalOutput")

    rounds = max(1, math.ceil(min(k_active, 48) / 8))
    t_idx = (k_active - 1) % 8

    with tile.TileContext(nc) as tc:
        with (
            tc.tile_pool(name="cpool", bufs=1) as cpool,
            tc.tile_pool(name="xpool", bufs=1) as xpool,
            tc.tile_pool(name="psum", bufs=1, space="PSUM") as pspool,
            tc.tile_pool(name="bpool", bufs=2) as bpool,
            tc.tile_pool(name="wpool", bufs=2) as wpool,
        ):
            # ---- tiles --------------------------------------------------
            ct_t = [[None] * (NJ + 1) for _ in range(2)]
            xa_t = {}
            xb_t = {}
            bc_t = [None] * 4

            def dma_ct_rs(h, jj):
                # rs-constant pair: only partition 0 carries data; memset the
                # rest on the (idle) Pool engine instead of streaming 256KB
                t = cpool.tile([128, 2, 1024], FP8, name=f"ctr{h}_{jj}",
                               tag=f"ctr{h}_{jj}")
                nc.gpsimd.memset(t[:], 0.0)
                nc.sync.dma_start(t[0:1], ct[h][0:1, jj])
                ct_t[h][jj] = t

            def dma_ct1(h, jj):
                t = cpool.tile([128, 2, 1024], FP8, name=f"cts{h}_{jj}",
                               tag=f"cts{h}_{jj}")
                nc.sync.dma_start(t[:], ct[h][:, jj])
                ct_t[h][jj] = t

            def dma_ct2(h, jp):
                t = cpool.tile([128, 2, 2, 1024], FP8, name=f"ct{h}_{jp}",
                               tag=f"ct{h}_{jp}")
                nc.sync.dma_start(t[:], ct[h][:, 2 * jp : 2 * jp + 2])
                ct_t[h][2 * jp] = t[:, 0]
                ct_t[h][2 * jp + 1] = t[:, 1]

            def dma_xa(m, p, eng=None):
                src = xhi if p == 0 else xlo
                t = xpool.tile([128, JA, 2, 128], FP8, name=f"xa{m}_{p}",
                               tag=f"xa{m}_{p}")
                (eng or nc.sync).dma_start(t[:], src[m][:, :JA])
                xa_t[(m, p)] = t

            def dma_xb(m, p):
                src = xhi if p == 0 else xlo
                t = xpool.tile([128, JB, 2, 128], FP8, name=f"xb{m}_{p}",
                               tag=f"xb{m}_{p}")
                nc.sync.dma_start(t[:], src[m][:, JA:])
                xb_t[(m, p)] = t

            def dma_bc(n):
                t = cpool.tile([128, 512], F32, name=f"bc{n}", tag=f"bc{n}")
                nc.sync.dma_start(t[:], bc[n])
                bc_t[n] = t

            def x_ap(m, p, j):
                if j < JA:
                    return xa_t[(m, p)][:, j]
                return xb_t[(m, p)][:, j - JA]

            # ---- DMA schedule (SP queue order = stream priority) --------
            dma_ct1(0, 0)
            dma_xa(0, 0)
            dma_xa(0, 1)
            dma_ct1(0, 1)
            dma_xa(1, 0)
            dma_ct2(0, 1)
            dma_xa(1, 1)
            dma_ct2(0, 2)
            dma_ct2(0, 3)
            dma_ct2(0, 4)
            dma_xb(0, 0)
            dma_ct2(0, 5)
            dma_xb(0, 1)
            dma_ct2(0, 6)
            dma_xb(1, 0)
            dma_ct2(0, 7)
            dma_xb(1, 1)
            dma_ct_rs(0, NPAIR)
            dma_ct_rs(0, NPAIR + 1)
            for jp in range(8):
                dma_ct2(1, jp)
                if jp == 5:
                    dma_bc(0)
                if jp == 7:
                    dma_bc(1)
            dma_ct_rs(1, NPAIR)
            dma_ct_rs(1, NPAIR + 1)
            dma_xa(2, 0)
            dma_xa(2, 1)
            dma_bc(2)
            dma_bc(3)
            dma_xb(2, 0)
            dma_xb(2, 1)
            dma_xa(3, 0)
            dma_xa(3, 1)
            dma_xb(3, 0)
            dma_xb(3, 1)

            # ---- compute helpers ---------------------------------------
            ps_t = {}

            def ps_tile(bs, m, gn, p):
                t = pspool.tile([128, 512], F32, name=f"ps{bs}_{gn % 2}_{p}",
                                tag=f"ps{bs}_{gn % 2}_{p}")
                ps_t[(m, gn, p)] = t
                return t

            def mm(m, gn, p, j, start, stop):
                h, n = gn // 2, gn % 2
                jj = j if j < NPAIR else (NPAIR if p == 0 else NPAIR + 1)
                rhs = ct_t[h][jj][:, :, n * 512 : (n + 1) * 512]
                nc.tensor.matmul(
                    ps_t[(m, gn, p)][:],
                    x_ap(m, p, j),
                    rhs,
                    start=start,
                    stop=stop,
                    perf_mode=DR,
                )

            u_t = {}
            boosted_t = {}
            cands_t = {}
            topm_t = {}
            MW = 8 * rounds + 128  # merge width for the tail tile

            def alloc_m(m):
                u_t[m] = bpool.tile([128, NCOL], F32, name="u", tag="u")
                boosted_t[m] = bpool.tile([128, NCOL], F32, name="boosted",
                                          tag="boosted", bufs=4)
                cands_t[m] = bpool.tile([128, 256], F32, name="cands",
                                        tag="cands", bufs=4)
                topm_t[m] = bpool.tile([128, MW], F32, name="topm",
                                       tag="topm")

            def combine(m, gn, merge=False, fast=False, mbase=16):
                # c1 (ACT): lo PSUM * 2^-6 -> SBUF
                c1 = wpool.tile([128, 512], F32, name=f"c1_{gn % 2}",
                                tag=f"c1_{gn % 2}")
                u = u_t[m][:, gn * 512 : (gn + 1) * 512]
                bslice = boosted_t[m][:, gn * 512 : (gn + 1) * 512]
                if fast:
                    # critical tail: shortest chain (ACT + DVE)
                    nc.scalar.mul(c1[:], ps_t[(m, gn, 1)][:], LO_SCALE)
                    nc.vector.tensor_tensor(u, ps_t[(m, gn, 0)][:], c1[:], ADD)
                    nc.vector.tensor_tensor(bslice, u, bc_t[gn][:], MULT)
                else:
                    # off critical path: zero DVE use. ACT drains both PSUM
                    # banks, Pool does the add and the boost multiply.
                    c0 = wpool.tile([128, 512], F32, name=f"c0_{gn % 2}",
                                    tag=f"c0_{gn % 2}")
                    nc.scalar.copy(c0[:], ps_t[(m, gn, 0)][:])
                    nc.scalar.mul(c1[:], ps_t[(m, gn, 1)][:], LO_SCALE)
                    nc.gpsimd.tensor_tensor(u, c0[:], c1[:], ADD)
                    nc.gpsimd.tensor_tensor(bslice, u, bc_t[gn][:], MULT)
                # segment top-8 candidates (8 segs of 64 per 512-chunk)
                for s in range(8):
                    seg = gn * 8 + s
                    if merge and seg >= mbase:
                        dst = topm_t[m][:, 8 * rounds + (seg - mbase) * 8 :
                                        8 * rounds + (seg - mbase + 1) * 8]
                    else:
                        dst = cands_t[m][:, seg * 8 : (seg + 1) * 8]
                    nc.vector.max(
                        dst, boosted_t[m][:, seg * 64 : (seg + 1) * 64])

            def rounds_chain(dst, src_full, width, wc_name):
                wc = wpool.tile([128, width], F32, name=wc_name, tag=wc_name,
                                bufs=1)
                src = src_full
                for r in range(rounds):
                    m8 = dst[:, r * 8 : (r + 1) * 8]
                    nc.vector.max(m8, src[:])
                    if r != rounds - 1:
                        nc.vector.match_replace(wc[:], m8, src[:], 0.0)
                        src = wc
                return dst[:, (rounds - 1) * 8 + t_idx :
                           (rounds - 1) * 8 + t_idx + 1]

            def finish_pre(m, w=128):
                # top-(8*rounds) of the first w/8 segments -> topm[:, :8*rounds]
                rounds_chain(topm_t[m][:, : 8 * rounds],
                             cands_t[m][:, :w], w, "wcp")

            def mask_out(m, split):
                bo = boosted_t[m]
                om = bpool.tile([128, NCOL], BF16, name="om", tag="om")
                thr = thr_t[m]
                if split:
                    nc.vector.scalar_tensor_tensor(
                        om[:, :HALF], bo[:, :HALF], thr, bo[:, :HALF],
                        ISGE, MULT)
                    nc.sync.dma_start(out[m][:, :HALF], om[:, :HALF])
                    nc.vector.scalar_tensor_tensor(
                        om[:, HALF:], bo[:, HALF:], thr, bo[:, HALF:],
                        ISGE, MULT)
                    nc.sync.dma_start(out[m][:, HALF:], om[:, HALF:])
                elif split is None:
                    # exact mask using only ACT + Pool (DVE-free):
                    # out = bo - bo * sign(relu(thr - bo))
                    for hh in range(2):
                        sl = slice(hh * HALF, (hh + 1) * HALF)
                        t2 = wpool.tile([128, HALF], F32, name="t2", tag="t2",
                                        bufs=1)
                        nc.scalar.activation(
                            t2[:], bo[:, sl],
                            mybir.ActivationFunctionType.Relu,
                            bias=thr, scale=-1.0)
                        s2 = wpool.tile([128, HALF], F32, name="s2", tag="s2",
                                        bufs=1)
                        nc.scalar.sign(s2[:], t2[:])
                        w2 = wpool.tile([128, HALF], F32, name="w2", tag="w2",
                                        bufs=1)
                        nc.gpsimd.tensor_tensor(w2[:], bo[:, sl], s2[:], MULT)
                        nc.gpsimd.tensor_tensor(
                            om[:, sl], bo[:, sl], w2[:],
                            mybir.AluOpType.subtract)
                        nc.sync.dma_start(out[m][:, sl], om[:, sl])
                else:
                    nc.vector.scalar_tensor_tensor(
                        om[:], bo[:], thr, bo[:], ISGE, MULT)
                    nc.sync.dma_start(out[m], om[:])

            thr_t = {}

            def rounds_simple(m):
                tops = wpool.tile([128, 8 * rounds], F32, name="tops",
                                  tag="tops")
                thr_t[m] = rounds_chain(tops, cands_t[m], 256, "wc")

            def rounds_merge(m, w=None):
                tops = wpool.tile([128, 8 * rounds], F32, name="topsm",
                                  tag="topsm")
                w = MW if w is None else w
                thr_t[m] = rounds_chain(tops, topm_t[m][:, :w], w, "wcm")

            def rounds_merge2(ma, mb):
                # two independent chains interleaved op-by-op so the DVE
                # never stalls on the intra-chain dependency latency
                tpa = wpool.tile([128, 8 * rounds], F32, name="tpa", tag="tpa", bufs=1)
                tpb = wpool.tile([128, 8 * rounds], F32, name="tpb", tag="tpb", bufs=1)
                wca = wpool.tile([128, MW], F32, name="wca", tag="wca", bufs=1)
                wcb = wpool.tile([128, MW], F32, name="wcb", tag="wcb", bufs=1)
                sa, sb = topm_t[ma], topm_t[mb]
                for r in range(rounds):
                    m8a = tpa[:, r * 8 : (r + 1) * 8]
                    m8b = tpb[:, r * 8 : (r + 1) * 8]
                    nc.vector.max(m8a, sa[:])
                    nc.vector.max(m8b, sb[:])
                    if r != rounds - 1:
                        nc.vector.match_replace(wca[:], m8a, sa[:], 0.0)
                        nc.vector.match_replace(wcb[:], m8b, sb[:], 0.0)
                        sa, sb = wca, wcb
                ti = (rounds - 1) * 8 + t_idx
                thr_t[ma] = tpa[:, ti : ti + 1]
                thr_t[mb] = tpb[:, ti : ti + 1]

            # ---- phases -------------------------------------------------
            # A: m0+m1 chunks (0,1) with ct half 0 streaming; B: chunks (2,3)
            for gns in ((0, 1), (2, 3)):
                for m in (0, 1):
                    for gn in gns:
                        for p in (0, 1):
                            ps_tile(m, m, gn, p)
                for j in range(NJ):
                    for m in (0, 1):
                        for p in (0, 1):
                            for gn in gns:
                                mm(m, gn, p, j, start=(j == 0), stop=(j == NJ - 1))
                if gns[0] == 0:
                    for m in (0, 1):
                        alloc_m(m)
                for m in (0, 1):
                    for gn in gns:
                        combine(m, gn, merge=(gns[0] == 2))
                if gns[0] == 0:
                    finish_pre(0)
                    finish_pre(1)
            # C: m2 as two j-outer pair groups on alternating bank sets
            # (m0/m1 finish emitted mid-C so their Pool masks don't block
            # m2's combine chain)
            alloc_m(2)
            # pair (0,1) j-outer (keeps xb-arrival slack at C start)
            for gn in (0, 1):
                for p in (0, 1):
                    ps_tile(0, 2, gn, p)
            for j in range(NJ):
                for p in (0, 1):
                    for gn in (0, 1):
                        mm(2, gn, p, j, start=(j == 0), stop=(j == NJ - 1))
            for gn in (0, 1):
                combine(2, gn)
            rounds_merge2(0, 1)
            # chunks 2,3 serial j-inner: combines start 3.6us earlier
            for gn in (2, 3):
                for p in (0, 1):
                    ps_tile(1, 2, gn, p)
                for p in (1, 0):
                    for j in range(NJ):
                        mm(2, gn, p, j, start=(j == 0), stop=(j == NJ - 1))
                combine(2, gn)
            rounds_simple(2)

            # D: m3 -> pair (0,1), then chunk 2, then chunk 3 (lo first);
            # pre-rounds overlap chunk-3 matmuls, merge rounds close the tail
            alloc_m(3)
            for gn in (0, 1):
                for p in (0, 1):
                    ps_tile(0, 3, gn, p)
                for p in (1, 0):
                    for j in range(NJ):
                        mm(3, gn, p, j, start=(j == 0), stop=(j == NJ - 1))
                combine(3, gn)
            finish_pre(3)
            mask_out(0, split=False)
            mask_out(1, split=False)
            for p in (0, 1):
                ps_tile(1, 3, 2, p)
            for p in (1, 0):
                for j in range(NJ):
                    mm(3, 2, p, j, start=(j == 0), stop=(j == NJ - 1))
            combine(3, 2, merge=True)
            mask_out(2, split=None)
            for p in (0, 1):
                ps_tile(1, 3, 3, p)
            for p in (1, 0):
                for j in range(NJ):
                    mm(3, 3, p, j, start=(j == 0), stop=(j == NJ - 1))
            combine(3, 3, merge=True, fast=True)
            rounds_merge(3)
            mask_out(3, split=True)
    nc.compile()
    return nc


def _get_nc(k_active: int):
    nc = _BUILD_CACHE.get(k_active)
    if nc is None:
        nc = _BUILD_CACHE[k_active] = _build(k_active)
    return nc


def _quant_split(y):
    """y (f32, [-0.5, 0.5)) -> (hi, lo) e4m3 arrays, y ~= hi + lo * 2^-6."""
    hi = y.astype(E4M3)
    lo = ((y - hi.astype(np.float32)) * np.float32(64.0)).astype(E4M3)
    return hi, lo


def _encode_rs(rs):
    """rs (f64 [2048], ~1024) -> 4 e4m3 rows (a, b, c, d) with
    8*a + 8*b + 4*(c + d) * 2^-6 ~= rs (max err ~4e-3).
    x-side constants: 8.0 on the hi-pass pair, 4.0 on the lo-pass pair."""
    a = (rs / 8.0).astype(E4M3)
    r1 = rs - 8.0 * a.astype(np.float64)
    b = (r1 / 8.0).astype(E4M3)
    r2 = r1 - 8.0 * b.astype(np.float64)
    c = (r2 * 16.0).astype(E4M3)
    r3 = r2 - 4.0 * c.astype(np.float64) / 64.0
    d = (r3 * 16.0).astype(E4M3)
    err = np.abs(r3 - 4.0 * d.astype(np.float64) / 64.0).max()
    assert err < 0.01, err
    return a, b, c, d


def kernel(input_vector, connections, boosting_factors, num_active):
    x = np.ascontiguousarray(input_vector, dtype=np.float32).reshape(-1, D)
    b = np.asarray(boosting_factors, dtype=np.float32)
    k = min(int(num_active), NCOL)
    # the segmented top-8-per-64 candidate scheme covers k <= 48 only
    assert 1 <= k <= 48, f"num_active={k} outside supported range"
    n_tok = x.shape[0]
    assert n_tok == N_CORES * TOK_PER_CORE, n_tok

    nc = _get_nc(k)

    # x^T centered, laid out [core, m, ks(part), kc, t], then e4m3 hi/lo split
    y = x - np.float32(0.5)
    yt = np.ascontiguousarray(y.T)                       # [D, n_tok]
    yt = yt.reshape(KC, 128, N_CORES, M_TILES, 128)      # [kc, ks, core, m, t]
    yt = yt.transpose(2, 3, 1, 0, 4)                     # [core, m, ks, kc, t]
    yt = np.ascontiguousarray(yt)
    yt_hi, yt_lo = _quant_split(yt)
    # append the rs-constant pair: partition 0 = 4.0 (hi) / 1.0 (lo)
    ext_hi = np.zeros((N_CORES, M_TILES, 128, 2, 128), dtype=E4M3)
    ext_lo = np.zeros((N_CORES, M_TILES, 128, 2, 128), dtype=E4M3)
    ext_hi[:, :, 0] = E4M3(8.0)
    ext_lo[:, :, 0] = E4M3(4.0)
    xt_hi = np.ascontiguousarray(
        np.concatenate([yt_hi.reshape(N_CORES, M_TILES, 128, NPAIR, 2, 128),
                        ext_hi[:, :, :, None]], axis=3))
    xt_lo = np.ascontiguousarray(
        np.concatenate([yt_lo.reshape(N_CORES, M_TILES, 128, NPAIR, 2, 128),
                        ext_lo[:, :, :, None]], axis=3))

    # C^T as [kc, ks, col] (exact in e4m3: 0/1), then [half, jj, ks, 2*1024]
    Cf = np.asarray(connections, dtype=np.float64)
    ctt = np.ascontiguousarray(Cf.T.astype(np.float32))  # [D, NCOL]
    ctt = ctt.reshape(KC, 128, NCOL)                     # [kc, ks, col]
    rs = 0.5 * Cf.sum(axis=1)                            # [NCOL]
    ra, rb, rc, rd = _encode_rs(rs)
    ct_full = np.zeros((2, 128, NJ + 1, 2, 1024), dtype=E4M3)
    for h in range(2):
        cols = slice(h * HALF, (h + 1) * HALF)
        for j in range(NPAIR):
            ct_full[h, :, j, 0] = ctt[2 * j][:, cols].astype(E4M3)
            ct_full[h, :, j, 1] = ctt[2 * j + 1][:, cols].astype(E4M3)
        ct_full[h, 0, NPAIR, 0] = ra[cols]
        ct_full[h, 0, NPAIR, 1] = rb[cols]
        ct_full[h, 0, NPAIR + 1, 0] = rc[cols]
        ct_full[h, 0, NPAIR + 1, 1] = rd[cols]

    bcast = np.ascontiguousarray(
        np.broadcast_to(b, (128, NCOL)).reshape(128, 4, 512).transpose(1, 0, 2))

    in_maps = [
        {"xhi": xt_hi[c], "xlo": xt_lo[c], "ct": ct_full, "bc": bcast}
        for c in range(N_CORES)
    ]
    res = run_bass_kernel_spmd(nc, in_maps, core_ids=list(range(N_CORES)))
    outs = [r["out"].reshape(TOK_PER_CORE, NCOL) for r in res.results]
    full = np.concatenate(outs, axis=0).astype(np.float32)
    return full.reshape(input_vector.shape[0], input_vector.shape[1], NCOL)


# revision 68
# speedup vs baseline: 1.0495x; 1.0495x over previous
"""HTM spatial-pooler kernel for Trainium2 (8 NeuronCores, data-parallel over tokens).

Computes, for x = input_vector reshaped to [4096 tokens, 4096]:
    overlap = x @ C^T               (C = connections [2048, 4096], binary)
    boosted = overlap * boost       (per-column boosting factors)
    masked  = where(boosted >= kth_largest_per_row(boosted, k), boosted, 0)

Strategy per core (512 tokens, 4 m-tiles of 128):
  - Matmul in fp8 (e4m3) with MatmulPerfMode.DoubleRow: each PE instruction
    contracts 256 (two 128-chunks) at 0.5 cycles/row -> 4x bf16 throughput.
  - Precision: x is centered (y = x - 0.5) and split y ~= hi + lo * 2^-6 with
    hi = e4m3(y), lo = e4m3((y - hi) * 64).  The lo pass accumulates into its
    own PSUM bank and is scaled by 2^-6 on the Activation engine while being
    moved to SBUF, then added to the hi PSUM on the DVE.  This gives ~7e-3
    score noise on scores whose top-40/41 gap is ~0.77 -> a handful of
    flipped rows (well inside the 2e-2 rel-err gate).
  - The centering constant (0.5 * popcount(C_k) per column) rides inside the
    matmul as an 18th DoubleRow pair: x-side is a constant column (4.0 for
    the hi pass, 1.0 for the lo pass) on partition 0, ct-side holds a 4-term
    e4m3 cascade encoding of the constant, so no extra DVE pass is needed.
  - Top-k threshold per row on the DVE (per-64-col top-8 candidates via
    max8, then max8/match_replace rounds; m0/m1/m3 use a pre-top-k over the
    first half plus a short merge round to shorten the critical tail).
    Masks are (boosted >= thr) * boosted written as bf16: one DVE
    scalar_tensor_tensor for m0/m1/m3, and an exact ACT+Pool construction
    (bo - bo * sign(relu(thr - bo))) for m2 to keep the DVE tail clear.
  - Engine budget: PE matmuls; ACT drains/rescales PSUM; Pool (GPSIMD,
    which can't touch PSUM and lacks TensorScalarPtr) takes the SBUF-side
    add/boost-multiply for non-critical chunks; DVE keeps the top-k and the
    critical-path combines.
  - Phase schedule: m0+m1 run column-half-at-a-time with j outermost so the
    PE consumes each ct pair-chunk as its DMA lands (ct streams 9.4 MB);
    m2/m3 run as j-outer pair groups / j-inner chunks with ct resident on
    alternating PSUM bank sets, completing chunk-eagerly so the
    combine/top-k pipeline hides behind the PE.
"""
import math

import numpy as np
import ml_dtypes

import concourse.bacc as bacc
import concourse.mybir as mybir
from concourse import tile
from concourse.bass_utils import run_bass_kernel_spmd

FP8 = mybir.dt.float8e4
BF16 = mybir.dt.bfloat16
F32 = mybir.dt.float32
E4M3 = ml_dtypes.float8_e4m3

N_CORES = 8
TOK_PER_CORE = 512
M_TILES = 4          # 128-token tiles per core
D = 4096             # input size (contraction)
KC = D // 128        # 32 contraction chunks
NPAIR = KC // 2      # 16 DoubleRow pairs of x data
NJ = NPAIR + 1       # +1 rs-constant pair
NCOL = 2048          # minicolumns
HALF = NCOL // 2
LO_SCALE = 2.0 ** -6

C3_ENG = [lambda nc: nc.gpsimd]  # Pool TT lowers to GPSIMD Multiply

_BUILD_CACHE = {}


def _build(k_active: int):
    nc = bacc.Bacc("TRN2", target_bir_lowering=False)
    DR = mybir.MatmulPerfMode.DoubleRow
    ADD = mybir.AluOpType.add
    MULT = mybir.AluOpType.mult
    ISGE = mybir.AluOpType.is_ge

    # x: per (m, pass): pairs j=0..16 (j=16 = rs constant column), split into
    # an early tile (j<9) and a late tile so the PE can start after ~3.3 us.
    JA, JB = 9, NJ - 9
    xhi = nc.dram_tensor("xhi", [M_TILES, 128, NJ, 2, 128], FP8, kind="ExternalInput")
    xlo = nc.dram_tensor("xlo", [M_TILES, 128, NJ, 2, 128], FP8, kind="ExternalInput")
    # ct: [half, jj, ks, 2*1024]; jj=16 -> hi-pass rs rows, jj=17 -> lo-pass
    ct = nc.dram_tensor("ct", [2, 128, NJ + 1, 2, 1024], FP8, kind="ExternalInput")
    bc = nc.dram_tensor("bc", [4, 128, 512], F32, kind="ExternalInput")
    out = nc.dram_tensor("out", [M_TILES, 128, NCOL], BF16, kind="ExternalOutput")

    rounds = max(1, math.ceil(min(k_active, 48) / 8))
    t_idx = (k_active - 1) % 8

    with tile.TileContext(nc) as tc:
        with (
            tc.tile_pool(name="cpool", bufs=1) as cpool,
            tc.tile_pool(name="xpool", bufs=1) as xpool,
            tc.tile_pool(name="psum", bufs=1, space="PSUM") as pspool,
            tc.tile_pool(name="bpool", bufs=2) as bpool,
            tc.tile_pool(name="wpool", bufs=2) as wpool,
        ):
            # ---- tiles --------------------------------------------------
            ct_t = [[None] * (NJ + 1) for _ in range(2)]
            xa_t = {}
            xb_t = {}
            bc_t = [None] * 4

            def dma_ct_rs(h, jj):
                # rs-constant pair: only partition 0 carries data; memset the
                # rest on the (idle) Pool engine instead of streaming 256KB
                t = cpool.tile([128, 2, 1024], FP8, name=f"ctr{h}_{jj}",
                               tag=f"ctr{h}_{jj}")
                nc.gpsimd.memset(t[:], 0.0)
                nc.sync.dma_start(t[0:1], ct[h][0:1, jj])
                ct_t[h][jj] = t

            def dma_ct1(h, jj):
                t = cpool.tile([128, 2, 1024], FP8, name=f"cts{h}_{jj}",
                               tag=f"cts{h}_{jj}")
                nc.sync.dma_start(t[:], ct[h][:, jj])
                ct_t[h][jj] = t

            def dma_ct2(h, jp):
                t = cpool.tile([128, 2, 2, 1024], FP8, name=f"ct{h}_{jp}",
                               tag=f"ct{h}_{jp}")
                nc.sync.dma_start(t[:], ct[h][:, 2 * jp : 2 * jp + 2])
                ct_t[h][2 * jp] = t[:, 0]
                ct_t[h][2 * jp + 1] = t[:, 1]

            def dma_xa(m, p, eng=None):
                src = xhi if p == 0 else xlo
                t = xpool.tile([128, JA, 2, 128], FP8, name=f"xa{m}_{p}",
                               tag=f"xa{m}_{p}")
                (eng or nc.sync).dma_start(t[:], src[m][:, :JA])
                xa_t[(m, p)] = t

            def dma_xb(m, p):
                src = xhi if p == 0 else xlo
                t = xpool.tile([128, JB, 2, 128], FP8, name=f"xb{m}_{p}",
                               tag=f"xb{m}_{p}")
                nc.sync.dma_start(t[:], src[m][:, JA:])
                xb_t[(m, p)] = t

            def dma_bc(n):
                t = cpool.tile([128, 512], F32, name=f"bc{n}", tag=f"bc{n}")
                nc.sync.dma_start(t[:], bc[n])
                bc_t[n] = t

            def x_ap(m, p, j):
                if j < JA:
                    return xa_t[(m, p)][:, j]
                return xb_t[(m, p)][:, j - JA]

            # ---- DMA schedule (SP queue order = stream priority) --------
            dma_ct1(0, 0)
            dma_xa(0, 0)
            dma_xa(0, 1)
            dma_ct1(0, 1)
            dma_xa(1, 0)
            dma_ct2(0, 1)
            dma_xa(1, 1)
            dma_ct2(0, 2)
            dma_ct2(0, 3)
            dma_ct2(0, 4)
            dma_xb(0, 0)
            dma_ct2(0, 5)
            dma_xb(0, 1)
            dma_ct2(0, 6)
            dma_xb(1, 0)
            dma_ct2(0, 7)
            dma_xb(1, 1)
            dma_ct_rs(0, NPAIR)
            dma_ct_rs(0, NPAIR + 1)
            for jp in range(8):
                dma_ct2(1, jp)
                if jp == 5:
                    dma_bc(0)
                if jp == 7:
                    dma_bc(1)
            dma_ct_rs(1, NPAIR)
            dma_ct_rs(1, NPAIR + 1)
            dma_xa(2, 0)
            dma_xa(2, 1)
            dma_bc(2)
            dma_bc(3)
            dma_xb(2, 0)
            dma_xb(2, 1)
            dma_xa(3, 0)
            dma_xa(3, 1)
            dma_xb(3, 0)
            dma_xb(3, 1)

            # ---- compute helpers ---------------------------------------
            ps_t = {}

            def ps_tile(bs, m, gn, p):
                t = pspool.tile([128, 512], F32, name=f"ps{bs}_{gn % 2}_{p}",
                                tag=f"ps{bs}_{gn % 2}_{p}")
                ps_t[(m, gn, p)] = t
                return t

            def mm(m, gn, p, j, start, stop):
                h, n = gn // 2, gn % 2
                jj = j if j < NPAIR else (NPAIR if p == 0 else NPAIR + 1)
                rhs = ct_t[h][jj][:, :, n * 512 : (n + 1) * 512]
                nc.tensor.matmul(
                    ps_t[(m, gn, p)][:],
                    x_ap(m, p, j),
                    rhs,
                    start=start,
                    stop=stop,
                    perf_mode=DR,
                )

            u_t = {}
            boosted_t = {}
            cands_t = {}
            topm_t = {}
            MW = 8 * rounds + 128  # merge width for the tail tile

            def alloc_m(m):
                u_t[m] = bpool.tile([128, NCOL], F32, name="u", tag="u")
                boosted_t[m] = bpool.tile([128, NCOL], F32, name="boosted",
                                          tag="boosted", bufs=4)
                cands_t[m] = bpool.tile([128, 256], F32, name="cands",
                                        tag="cands", bufs=4)
                topm_t[m] = bpool.tile([128, MW], F32, name="topm",
                                       tag="topm")

            def combine(m, gn, merge=False, fast=False, mbase=16):
                # c1 (ACT): lo PSUM * 2^-6 -> SBUF
                c1 = wpool.tile([128, 512], F32, name=f"c1_{gn % 2}",
                                tag=f"c1_{gn % 2}")
                u = u_t[m][:, gn * 512 : (gn + 1) * 512]
                bslice = boosted_t[m][:, gn * 512 : (gn + 1) * 512]
                if fast:
                    # critical tail: shortest chain (ACT + DVE)
                    nc.scalar.mul(c1[:], ps_t[(m, gn, 1)][:], LO_SCALE)
                    nc.vector.tensor_tensor(u, ps_t[(m, gn, 0)][:], c1[:], ADD)
                    nc.vector.tensor_tensor(bslice, u, bc_t[gn][:], MULT)
                else:
                    # off critical path: zero DVE use. ACT drains both PSUM
                    # banks, Pool does the add and the boost multiply.
                    c0 = wpool.tile([128, 512], F32, name=f"c0_{gn % 2}",
                                    tag=f"c0_{gn % 2}")
                    nc.scalar.copy(c0[:], ps_t[(m, gn, 0)][:])
                    nc.scalar.mul(c1[:], ps_t[(m, gn, 1)][:], LO_SCALE)
                    nc.gpsimd.tensor_tensor(u, c0[:], c1[:], ADD)
                    nc.gpsimd.tensor_tensor(bslice, u, bc_t[gn][:], MULT)
                # segment top-8 candidates (8 segs of 64 per 512-chunk)
                for s in range(8):
                    seg = gn * 8 + s
                    if merge and seg >= mbase:
                        dst = topm_t[m][:, 8 * rounds + (seg - mbase) * 8 :
                                        8 * rounds + (seg - mbase + 1) * 8]
                    else:
                        dst = cands_t[m][:, seg * 8 : (seg + 1) * 8]
                    nc.vector.max(
                        dst, boosted_t[m][:, seg * 64 : (seg + 1) * 64])

            def rounds_chain(dst, src_full, width, wc_name):
                wc = wpool.tile([128, width], F32, name=wc_name, tag=wc_name,
                                bufs=1)
                src = src_full
                for r in range(rounds):
                    m8 = dst[:, r * 8 : (r + 1) * 8]
                    nc.vector.max(m8, src[:])
                    if r != rounds - 1:
                        nc.vector.match_replace(wc[:], m8, src[:], 0.0)
                        src = wc
                return dst[:, (rounds - 1) * 8 + t_idx :
                           (rounds - 1) * 8 + t_idx + 1]

            def finish_pre(m, w=128):
                # top-(8*rounds) of the first w/8 segments -> topm[:, :8*rounds]
                rounds_chain(topm_t[m][:, : 8 * rounds],
                             cands_t[m][:, :w], w, "wcp")

            def mask_out(m, split):
                bo = boosted_t[m]
                om = bpool.tile([128, NCOL], BF16, name="om", tag="om")
                thr = thr_t[m]
                if split:
                    nc.vector.scalar_tensor_tensor(
                        om[:, :HALF], bo[:, :HALF], thr, bo[:, :HALF],
                        ISGE, MULT)
                    nc.sync.dma_start(out[m][:, :HALF], om[:, :HALF])
                    nc.vector.scalar_tensor_tensor(
                        om[:, HALF:], bo[:, HALF:], thr, bo[:, HALF:],
                        ISGE, MULT)
                    nc.sync.dma_start(out[m][:, HALF:], om[:, HALF:])
                elif split is None:
                    # exact mask using only ACT + Pool (DVE-free):
                    # out = bo - bo * sign(relu(thr - bo))
                    for hh in range(2):
                        sl = slice(hh * HALF, (hh + 1) * HALF)
                        t2 = wpool.tile([128, HALF], F32, name="t2", tag="t2",
                                        bufs=1)
                        nc.scalar.activation(
                            t2[:], bo[:, sl],
                            mybir.ActivationFunctionType.Relu,
                            bias=thr, scale=-1.0)
                        s2 = wpool.tile([128, HALF], F32, name="s2", tag="s2",
                                        bufs=1)
                        nc.scalar.sign(s2[:], t2[:])
                        w2 = wpool.tile([128, HALF], F32, name="w2", tag="w2",
                                        bufs=1)
                        nc.gpsimd.tensor_tensor(w2[:], bo[:, sl], s2[:], MULT)
                        nc.gpsimd.tensor_tensor(
                            om[:, sl], bo[:, sl], w2[:],
                            mybir.AluOpType.subtract)
                        nc.sync.dma_start(out[m][:, sl], om[:, sl])
                else:
                    nc.vector.scalar_tensor_tensor(
                        om[:], bo[:], thr, bo[:], ISGE, MULT)
                    nc.sync.dma_start(out[m], om[:])

            thr_t = {}

            def rounds_simple(m):
                tops = wpool.tile([128, 8 * rounds], F32, name="tops",
                                  tag="tops")
                thr_t[m] = rounds_chain(tops, cands_t[m], 256, "wc")

            def rounds_merge(m, w=None):
                tops = wpool.tile([128, 8 * rounds], F32, name="topsm",
                                  tag="topsm")
                w = MW if w is None else w
                thr_t[m] = rounds_chain(tops, topm_t[m][:, :w], w, "wcm")

            def rounds_merge2(ma, mb):
                # two independent chains interleaved op-by-op so the DVE
                # never stalls on the intra-chain dependency latency
                tpa = wpool.tile([128, 8 * rounds], F32, name="tpa", tag="tpa", bufs=1)
                tpb = wpool.tile([128, 8 * rounds], F32, name="tpb", tag="tpb", bufs=1)
                wca = wpool.tile([128, MW], F32, name="wca", tag="wca", bufs=1)
                wcb = wpool.tile([128, MW], F32, name="wcb", tag="wcb", bufs=1)
                sa, sb = topm_t[ma], topm_t[mb]
                for r in range(rounds):
                    m8a = tpa[:, r * 8 : (r + 1) * 8]
                    m8b = tpb[:, r * 8 : (r + 1) * 8]
                    nc.vector.max(m8a, sa[:])
                    nc.vector.max(m8b, sb[:])
                    if r != rounds - 1:
                        nc.vector.match_replace(wca[:], m8a, sa[:], 0.0)
                        nc.vector.match_replace(wcb[:], m8b, sb[:], 0.0)
                        sa, sb = wca, wcb
                ti = (rounds - 1) * 8 + t_idx
                thr_t[ma] = tpa[:, ti : ti + 1]
                thr_t[mb] = tpb[:, ti : ti + 1]

            # ---- phases -------------------------------------------------
            # A: m0+m1 chunks (0,1) with ct half 0 streaming; B: chunks (2,3)
            for gns in ((0, 1), (2, 3)):
                for m in (0, 1):
                    for gn in gns:
                        for p in (0, 1):
                            ps_tile(m, m, gn, p)
                for j in range(NJ):
                    for m in (0, 1):
                        for p in (0, 1):
                            for gn in gns:
                                mm(m, gn, p, j, start=(j == 0), stop=(j == NJ - 1))
                if gns[0] == 0:
                    for m in (0, 1):
                        alloc_m(m)
                for m in (0, 1):
                    for gn in gns:
                        combine(m, gn, merge=(gns[0] == 2))
                if gns[0] == 0:
                    finish_pre(0)
                    finish_pre(1)
            # C: m2 as two j-outer pair groups on alternating bank sets
            # (m0/m1 finish emitted mid-C so their Pool masks don't block
            # m2's combine chain)
            alloc_m(2)
            # pair (0,1) j-outer (keeps xb-arrival slack at C start)
            for gn in (0, 1):
                for p in (0, 1):
                    ps_tile(0, 2, gn, p)
            for j in range(NJ):
                for p in (0, 1):
                    for gn in (0, 1):
                        mm(2, gn, p, j, start=(j == 0), stop=(j == NJ - 1))
            for gn in (0, 1):
                combine(2, gn)
            rounds_merge2(0, 1)
            # chunks 2,3 serial j-inner: combines start 3.6us earlier
            for gn in (2, 3):
                for p in (0, 1):
                    ps_tile(1, 2, gn, p)
                for p in (1, 0):
                    for j in range(NJ):
                        mm(2, gn, p, j, start=(j == 0), stop=(j == NJ - 1))
                combine(2, gn)
            rounds_simple(2)

            # D: m3 -> pair (0,1), then chunk 2, then chunk 3 (lo first);
            # pre-rounds overlap chunk-3 matmuls, merge rounds close the tail
            alloc_m(3)
            for gn in (0, 1):
                for p in (0, 1):
                    ps_tile(0, 3, gn, p)
                for p in (1, 0):
                    for j in range(NJ):
                        mm(3, gn, p, j, start=(j == 0), stop=(j == NJ - 1))
                combine(3, gn)
            finish_pre(3)
            mask_out(0, split=False)
            mask_out(1, split=False)
            mask_out(2, split=None)
            for p in (0, 1):
                ps_tile(1, 3, 2, p)
            for p in (1, 0):
                for j in range(NJ):
                    mm(3, 2, p, j, start=(j == 0), stop=(j == NJ - 1))
            combine(3, 2, merge=True, fast=True)
            for p in (0, 1):
                ps_tile(1, 3, 3, p)
            for p in (1, 0):
                for j in range(NJ):
                    mm(3, 3, p, j, start=(j == 0), stop=(j == NJ - 1))
            combine(3, 3, merge=True, fast=True)
            rounds_merge(3)
            mask_out(3, split=True)
    nc.compile()
    return nc


def _get_nc(k_active: int):
    nc = _BUILD_CACHE.get(k_active)
    if nc is None:
        nc = _BUILD_CACHE[k_active] = _build(k_active)
    return nc


def _quant_split(y):
    """y (f32, [-0.5, 0.5)) -> (hi, lo) e4m3 arrays, y ~= hi + lo * 2^-6."""
    hi = y.astype(E4M3)
    lo = ((y - hi.astype(np.float32)) * np.float32(64.0)).astype(E4M3)
    return hi, lo


def _encode_rs(rs):
    """rs (f64 [2048], ~1024) -> 4 e4m3 rows (a, b, c, d) with
    8*a + 8*b + 4*(c + d) * 2^-6 ~= rs (max err ~4e-3).
    x-side constants: 8.0 on the hi-pass pair, 4.0 on the lo-pass pair."""
    a = (rs / 8.0).astype(E4M3)
    r1 = rs - 8.0 * a.astype(np.float64)
    b = (r1 / 8.0).astype(E4M3)
    r2 = r1 - 8.0 * b.astype(np.float64)
    c = (r2 * 16.0).astype(E4M3)
    r3 = r2 - 4.0 * c.astype(np.float64) / 64.0
    d = (r3 * 16.0).astype(E4M3)
    err = np.abs(r3 - 4.0 * d.astype(np.float64) / 64.0).max()
    assert err < 0.01, err
    return a, b, c, d


def kernel(input_vector, connections, boosting_factors, num_active):
    x = np.ascontiguousarray(input_vector, dtype=np.float32).reshape(-1, D)
    b = np.asarray(boosting_factors, dtype=np.float32)
    k = min(int(num_active), NCOL)
    # the segmented top-8-per-64 candidate scheme covers k <= 48 only
    assert 1 <= k <= 48, f"num_active={k} outside supported range"
    n_tok = x.shape[0]
    assert n_tok == N_CORES * TOK_PER_CORE, n_tok

    nc = _get_nc(k)

    # x^T centered, laid out [core, m, ks(part), kc, t], then e4m3 hi/lo split
    y = x - np.float32(0.5)
    yt = np.ascontiguousarray(y.T)                       # [D, n_tok]
    yt = yt.reshape(KC, 128, N_CORES, M_TILES, 128)      # [kc, ks, core, m, t]
    yt = yt.transpose(2, 3, 1, 0, 4)                     # [core, m, ks, kc, t]
    yt = np.ascontiguousarray(yt)
    yt_hi, yt_lo = _quant_split(yt)
    # append the rs-constant pair: partition 0 = 4.0 (hi) / 1.0 (lo)
    ext_hi = np.zeros((N_CORES, M_TILES, 128, 2, 128), dtype=E4M3)
    ext_lo = np.zeros((N_CORES, M_TILES, 128, 2, 128), dtype=E4M3)
    ext_hi[:, :, 0] = E4M3(8.0)
    ext_lo[:, :, 0] = E4M3(4.0)
    xt_hi = np.ascontiguousarray(
        np.concatenate([yt_hi.reshape(N_CORES, M_TILES, 128, NPAIR, 2, 128),
                        ext_hi[:, :, :, None]], axis=3))
    xt_lo = np.ascontiguousarray(
        np.concatenate([yt_lo.reshape(N_CORES, M_TILES, 128, NPAIR, 2, 128),
                        ext_lo[:, :, :, None]], axis=3))

    # C^T as [kc, ks, col] (exact in e4m3: 0/1), then [half, jj, ks, 2*1024]
    Cf = np.asarray(connections, dtype=np.float64)
    ctt = np.ascontiguousarray(Cf.T.astype(np.float32))  # [D, NCOL]
    ctt = ctt.reshape(KC, 128, NCOL)                     # [kc, ks, col]
    rs = 0.5 * Cf.sum(axis=1)                            # [NCOL]
    ra, rb, rc, rd = _encode_rs(rs)
    ct_full = np.zeros((2, 128, NJ + 1, 2, 1024), dtype=E4M3)
    for h in range(2):
        cols = slice(h * HALF, (h + 1) * HALF)
        for j in range(NPAIR):
            ct_full[h, :, j, 0] = ctt[2 * j][:, cols].astype(E4M3)
            ct_full[h, :, j, 1] = ctt[2 * j + 1][:, cols].astype(E4M3)
        ct_full[h, 0, NPAIR, 0] = ra[cols]
        ct_full[h, 0, NPAIR, 1] = rb[cols]
        ct_full[h, 0, NPAIR + 1, 0] = rc[cols]
        ct_full[h, 0, NPAIR + 1, 1] = rd[cols]

    bcast = np.ascontiguousarray(
        np.broadcast_to(b, (128, NCOL)).reshape(128, 4, 512).transpose(1, 0, 2))

    in_maps = [
        {"xhi": xt_hi[c], "xlo": xt_lo[c], "ct": ct_full, "bc": bcast}
        for c in range(N_CORES)
    ]
    res = run_bass_kernel_spmd(nc, in_maps, core_ids=list(range(N_CORES)))
    outs = [r["out"].reshape(TOK_PER_CORE, NCOL) for r in res.results]
    full = np.concatenate(outs, axis=0).astype(np.float32)
    return full.reshape(input_vector.shape[0], input_vector.shape[1], NCOL)
